# revision 10
# baseline (speedup 1.0000x reference)
"""Multi-head causal attention (B=2, S=2048, d_model=1024, H=16) on 8 trn2 cores.

Sharding: core c handles batch b=c//4 and the 4 heads g=c%4 -> heads [4g, 4g+4).
Host sums the 4 per-batch partial W_O products.

v2 design (cost-model-driven):
  - Everything ships/stores fp16 (same PE rate as bf16, 8x the mantissa).
  - ST scores stay [sk, sq] with exact-causal ranges.
  - AV runs NON-transposed: out[q(128), 65] per q-block with V carrying a
    ones column (col 64 = softmax denominator).  Full 128-partition use makes
    AV ~2x cheaper than the transposed form.
  - Causal triangles are masked on the PE: fp8 DoubleRow matmuls accumulate a
    -224 triangle (via a tri constant x shifted-identity windows) into the
    score PSUM, so no vector-engine masking at all.
  - Normalization: one reciprocal [128,4] per (head, chunk), then per-block
    tensor_scalar multiplies (DVE) writing fp16 o2 tiles.
  - o2 [q,d] -> oT [d,q] via XBAR DMA transpose (zero engine cost).
  - W_O from oT2 fp16; output written fp16, host upcasts and sums partials.
  - W_O of chunk c is deferred two chunks and pinned (scheduling edges) into
    the exp-bound windows of later chunks, front-loaded onto early heads.
  - PSUM banks: ST pairs 2x[128,1024] (4) + AV 1x[128,260] (1) +
    proj 1x[128,512] (1) + WO 2x[128,512] (2) = 8.
"""

import sys

for _p in ("/opt/trn_rl_repo",):
    if _p not in sys.path:
        sys.path.append(_p)

import numpy as np

import concourse.bass as bass
import concourse.mybir as mybir
import concourse.tile as tile
from concourse.tile import add_dep_helper
from concourse.vector_clock import ScopedClock
import bass_rust

# ---------------------------------------------------------------- constants
B = 2
S = 2048
D = 1024
H = 16
DK = 64
HPC = 4           # heads per core
E = HPC * DK      # 256 output dims per core
NB = S // 128     # 16 sk blocks
NCH = S // 512    # 4 sq chunks
DCH = D // 128    # 8 contraction chunks
DC2 = D // 256    # 4 fp8-DoubleRow contraction chunks
F32 = mybir.dt.float32
F16 = mybir.dt.float16
F8 = mybir.dt.float8e4
NEGM = -224.0     # causal mask addend (exp((s-224)/8) ~ 1e-10; e4m3-exact)
WSC_QK = 16.0     # host scale on W_Q/W_K before e4m3 split (st gains 256x)
WSC_V = 32.0      # host scale on W_V (folded back out via W_O/32)
EXP_SCALE = 0.125 / (WSC_QK * WSC_QK)


class _SplitWaitTileContext(tile.TileContext):
    """TileContext that carries at most one semaphore wait per emitted
    instruction (the walrus in this container rejects multi-wait
    instructions): extra waits are hoisted onto same-engine NOPs inserted
    immediately before the instruction."""

    N_PRE_NOPS = 10
    _waitnop_counter = 0

    def _lower_ordered_insts(self, ordered):
        for bbname, insts in ordered.items():
            new_list = []
            for inst in insts:
                si = getattr(inst, "sync_info", None)
                eng = getattr(inst, "engine", None)
                if si is not None and eng is not None and len(si.on_wait) > 1:
                    waits = list(si.on_wait)
                    *pre, last = waits
                    for w in pre:
                        _SplitWaitTileContext._waitnop_counter += 1
                        nop = mybir.InstNoOp(
                            name=f"waitnop-{self.uid}-{self._waitnop_counter}",
                            sync_info=mybir.SyncInfo(on_wait=[w], on_update=[]),
                            bass_nofuse=True,
                            engine=eng,
                        )
                        self.nc.register_instruction(nop, overwrite=True)
                        new_list.append(nop)
                    inst.sync_info = mybir.SyncInfo(
                        on_wait=[last], on_update=list(si.on_update)
                    )
                new_list.append(inst)
            ordered[bbname] = new_list
        return super()._lower_ordered_insts(ordered)

    def _drain_and_barrier(self, tick_clock, wait_clock):
        nops = [self.nc.sync.nop() for _ in range(self.N_PRE_NOPS)]
        drain_inst = self.nc.sync.drain()
        wait_clock.add_sem_waits(
            drain_inst.ins, ScopedClock({None: tick_clock.global_clock})
        )
        si = drain_inst.ins.sync_info
        waits = list(si.on_wait) if si is not None else []
        if len(waits) > 1:
            *pre, last = waits
            assert len(pre) <= len(nops), f"too many drain waits: {len(waits)}"
            for nop_bi, w in zip(nops, pre):
                nop_bi.ins.sync_info = bass_rust.SyncInfo(on_wait=[w], on_update=[])
            drain_inst.ins.sync_info = bass_rust.SyncInfo(
                on_wait=[last], on_update=list(si.on_update)
            )
        self.nc.all_engine_barrier()
        popped = self.nc._tile_sem_poison_stack.pop()
        assert popped is self._sem_poison
        self.nc.clear_and_free_semaphores(list(self.sems.allocated().values()))
        self.nc.all_engine_barrier()


def build_module() -> bass.Bass:
    nc = bass.Bass()

    # x pre-split to e4m3 hi/lo: [p, c4, hl, dc2, r, s] with
    # d = dc2*256 + r*128 + p; weights [p, hl, dc2, r, e] (e4m3, host-scaled).
    xqT = nc.dram_tensor("xqT", [128, NCH, 2, DC2, 2, 512], F8, kind="ExternalInput")
    xkT = nc.dram_tensor("xkT", [128, NCH, 2, DC2, 2, 512], F8, kind="ExternalInput")
    xvT = nc.dram_tensor("xvT", [128, NCH, 2, DC2, 2, 512], F8, kind="ExternalInput")
    wqT = nc.dram_tensor("wqT", [128, 2, DC2, 2, E], F8, kind="ExternalInput")
    wkT = nc.dram_tensor("wkT", [128, 2, DC2, 2, E], F8, kind="ExternalInput")
    wvT = nc.dram_tensor("wvT", [128, 2, DC2, 2, E], F8, kind="ExternalInput")
    woT = nc.dram_tensor("woT", [128, 2, D], F16, kind="ExternalInput")
    # TRI[r, m] = -224 if m >= r else 0, on BOTH DoubleRow sub-rows; paired
    # with 128-valued cw windows this adds -224*256 to the 256x-scaled st.
    triC = nc.dram_tensor("triC", [128, 2, 128], F8, kind="ExternalInput")
    # CW: cols 0..511 = e_0 rows (prefix), cols 512..639 = shifted identity
    cwC = nc.dram_tensor("cwC", [128, 2, 640], F8, kind="ExternalInput")
    out = nc.dram_tensor("out", [S, D], F16, kind="ExternalOutput")

    with _SplitWaitTileContext(nc) as tc:
        with (
            nc.allow_low_precision(reason="fp16 value path; fp32 PSUM accumulate"),
            tc.tile_pool(name="persist", bufs=1) as persist,
            tc.tile_pool(name="xin", bufs=6) as xin,
            tc.tile_pool(name="expool", bufs=4) as expool,
            tc.tile_pool(name="o2pool", bufs=16) as o2pool,
            tc.tile_pool(name="recpool", bufs=4) as recpool,
            tc.tile_pool(name="obpool", bufs=7) as obpool,
            tc.tile_pool(name="psA", bufs=2, space="PSUM") as psA,
            tc.tile_pool(name="psB", bufs=1, space="PSUM") as psB,
            tc.tile_pool(name="psC", bufs=1, space="PSUM") as psC,
            tc.tile_pool(name="psE", bufs=2, space="PSUM") as psE,
        ):
            # ---------------- resident tensors
            wq_sb = persist.tile([128, 2, DC2, 2, E], F8)
            wk_sb = persist.tile([128, 2, DC2, 2, E], F8)
            wv_sb = persist.tile([128, 2, DC2, 2, E], F8)
            wo_sb = persist.tile([128, 2, D], F16)
            tri_sb = persist.tile([128, 2, 128], F8)
            cw_sb = persist.tile([128, 2, 640], F8)
            qT2 = persist.tile([128, 2, S], F16)    # [dk-pair, t, sq]
            kT2 = persist.tile([128, 2, S], F16)
            V4 = persist.tile([128, NB, HPC * 65], F16)
            oT2 = persist.tile([128, 2, S], F16)    # [d-pair, t, sq]

            V4v = V4.rearrange("p n (h e) -> p n h e", h=HPC)

            xt_tiles = {}

            # (x_hl, w_hl) product terms: hi*Whi + hi*Wlo + lo*Whi; x_lo is
            # only needed for the last 4 matmuls of each group.
            TERMS = ((0, 0), (0, 1), (1, 0))

            def x_dma(name, c4, eng=None):
                xsrc = {"q": xqT, "k": xkT, "v": xvT}[name]
                xt = xin.tile([128, 2, DC2, 2, 512], F8, tag="xt", name=f"xt_{name}{c4}")
                for hl in range(2):
                    (eng or nc.sync).dma_start(
                        out=xt[:, hl], in_=xsrc[:, c4, hl]
                    )
                xt_tiles[(name, c4)] = xt

            def proj_group(name, c4, g):
                """One projection PSUM group: q/k -> eb half g; v -> s-block g."""
                xt = xt_tiles[(name, c4)]
                ps = psC.tile([128, 512], F32, tag="proj", name=f"pj_{name}{c4}{g}")
                if name != "v":
                    wsb = wq_sb if name == "q" else wk_sb
                    dst = qT2 if name == "q" else kT2
                    for ti, (xh, wh) in enumerate(TERMS):
                        for dc2 in range(DC2):
                            nc.tensor.matmul(
                                ps[:, 0:512],
                                wsb[:, wh, dc2, :, 128 * g : 128 * (g + 1)],
                                xt[:, xh, dc2, :, :],
                                start=(ti == 0 and dc2 == 0),
                                stop=(ti == 2 and dc2 == DC2 - 1),
                                perf_mode=mybir.MatmulPerfMode.DoubleRow,
                            )
                    nc.vector.tensor_copy(
                        out=dst[:, g, 512 * c4 : 512 * (c4 + 1)], in_=ps[:, 0:512]
                    )
                else:
                    j = 4 * c4 + g
                    for ti, (xh, wh) in enumerate(TERMS):
                        for dc2 in range(DC2):
                            nc.tensor.matmul(
                                ps[:, 0:E],
                                xt[:, xh, dc2, :, 128 * g : 128 * (g + 1)],
                                wv_sb[:, wh, dc2, :, :],
                                start=(ti == 0 and dc2 == 0),
                                stop=(ti == 2 and dc2 == DC2 - 1),
                                perf_mode=mybir.MatmulPerfMode.DoubleRow,
                            )
                    nc.vector.tensor_copy(
                        out=V4v[:, j, :, 0:64],
                        in_=ps[:, 0:E].rearrange("p (h e) -> p h e", h=HPC),
                    )

            o2_tiles = {}
            anchors = {}

            def attention(h, c4, filler=None):
                """Head h over query chunk c4 -> o2 tiles (and XBAR on h odd).
                filler() is called between sk-block pairs to interleave
                independent PE work (deferred W_O groups) into the queue."""
                t, r0 = h // 2, 64 * (h % 2)
                sqlo = 512 * c4
                av = psB.tile([128, HPC * 65], F32, tag="av", name=f"av_{h}_{c4}")
                avv = av.rearrange("p (m c) -> p m c", c=65)
                av_first = [True]
                last_pair = 4 * c4 + 2

                for jp in range(0, 4 * c4 + 4, 2):
                    if filler is not None and jp > 0:
                        filler()
                    i0 = jp - 4 * c4
                    st = psA.tile([128, 1024], F32, tag="st", name=f"st_{h}_{c4}_{jp}")
                    # ---- bank A: block jp
                    lo0 = 128 * max(i0, 0)
                    bi = nc.tensor.matmul(
                        st[:, lo0:512],
                        kT2[r0 : r0 + 64, t, 128 * jp : 128 * (jp + 1)],
                        qT2[r0 : r0 + 64, t, sqlo + lo0 : sqlo + 512],
                        start=True,
                        stop=(i0 < 0),
                    )
                    if jp == 0:
                        anchors[(h, c4)] = bi.ins
                    if i0 >= 0:
                        nc.tensor.matmul(
                            st[:, lo0 : lo0 + 128],
                            tri_sb[:, :, :],
                            cw_sb[:, :, 512:640],
                            start=False,
                            stop=True,
                            skip_group_check=True,
                            perf_mode=mybir.MatmulPerfMode.DoubleRow,
                        )
                    # ---- bank B: block jp+1
                    j1 = jp + 1
                    i1 = i0 + 1
                    if i1 <= 0:
                        nc.tensor.matmul(
                            st[:, 512:1024],
                            kT2[r0 : r0 + 64, t, 128 * j1 : 128 * (j1 + 1)],
                            qT2[r0 : r0 + 64, t, sqlo : sqlo + 512],
                            start=True,
                            stop=True,
                        )
                    elif i1 == 1:
                        # pair 1: one contiguous exp later; -1000 the 128-wide
                        # dead prefix so exp'ing it is harmless.
                        pw = 128
                        nc.tensor.matmul(
                            st[:, 512 : 512 + pw],
                            tri_sb[:, :, :],
                            cw_sb[:, :, 0:pw],
                            start=True,
                            stop=False,
                            skip_group_check=True,
                            perf_mode=mybir.MatmulPerfMode.DoubleRow,
                        )
                        nc.tensor.matmul(
                            st[:, 512 + pw : 1024],
                            kT2[r0 : r0 + 64, t, 128 * j1 : 128 * (j1 + 1)],
                            qT2[r0 : r0 + 64, t, sqlo + pw : sqlo + 512],
                            start=False,
                            stop=False,
                            skip_group_check=True,
                        )
                        nc.tensor.matmul(
                            st[:, 512 + pw : 512 + pw + 128],
                            tri_sb[:, :, :],
                            cw_sb[:, :, 512:640],
                            start=False,
                            stop=True,
                            skip_group_check=True,
                            perf_mode=mybir.MatmulPerfMode.DoubleRow,
                        )
                    else:
                        # pair 2 (i1 == 3): exact-range exps, no prefix fill
                        pw = 128 * i1
                        nc.tensor.matmul(
                            st[:, 512 + pw : 1024],
                            kT2[r0 : r0 + 64, t, 128 * j1 : 128 * (j1 + 1)],
                            qT2[r0 : r0 + 64, t, sqlo + pw : sqlo + 512],
                            start=True,
                            stop=False,
                        )
                        nc.tensor.matmul(
                            st[:, 512 + pw : 512 + pw + 128],
                            tri_sb[:, :, :],
                            cw_sb[:, :, 512:640],
                            start=False,
                            stop=True,
                            skip_group_check=True,
                            perf_mode=mybir.MatmulPerfMode.DoubleRow,
                        )
                    # ---- exp
                    ex = expool.tile(
                        [128, 1024], F16, tag="ex", name=f"ex_{h}_{c4}_{jp}"
                    )
                    if i0 <= 0:
                        nc.scalar.activation(
                            out=ex[:, lo0:1024],
                            in_=st[:, lo0:1024],
                            func=mybir.ActivationFunctionType.Exp,
                            scale=EXP_SCALE,
                        )
                    else:
                        nc.scalar.activation(
                            out=ex[:, lo0:512],
                            in_=st[:, lo0:512],
                            func=mybir.ActivationFunctionType.Exp,
                            scale=EXP_SCALE,
                        )
                        nc.scalar.activation(
                            out=ex[:, 512 + 128 * i1 : 1024],
                            in_=st[:, 512 + 128 * i1 : 1024],
                            func=mybir.ActivationFunctionType.Exp,
                            scale=EXP_SCALE,
                        )
                    # ---- AV (non-transposed): per live (q-block, sk-block)
                    for u in range(2):
                        j = jp + u
                        for im in range(HPC):
                            m = 4 * c4 + im
                            if j > m:
                                continue
                            nc.tensor.matmul(
                                avv[:, im, :],
                                ex[:, 512 * u + 128 * im : 512 * u + 128 * im + 128],
                                V4[:, j, 65 * h : 65 * (h + 1)],
                                start=av_first[0],
                                stop=(jp == last_pair and u == 1 and im == HPC - 1),
                                skip_group_check=True,
                            )
                            av_first[0] = False

                av_tiles[(h, c4)] = av
                normalize(h, c4)
                if h % 2 == 1:
                    xbar(h // 2, c4)

            av_tiles = {}

            def normalize(h, c4):
                """Drain av(h, c4) -> o2 fp16.  Emitted one head late so the
                semaphore waits are already satisfied when DVE/Pool reach them
                (no head-of-line blocking of proj copies behind)."""
                t, r0 = h // 2, 64 * (h % 2)
                av = av_tiles.pop((h, c4))
                avv = av.rearrange("p (m c) -> p m c", c=65)
                rec = recpool.tile([128, HPC], F32, tag="rec", name=f"rec_{h}_{c4}")
                nc.vector.reciprocal(out=rec, in_=avv[:, :, 64])
                for im in range(HPC):
                    # gpsimd cannot read PSUM; at the very tail ACT is idle,
                    # so split the last head's mults DVE/ACT to halve latency
                    use_act = h == 3 and c4 == NCH - 1 and im >= 2
                    key = (t, im)
                    if h % 2 == 0:
                        o2_tiles[key] = o2pool.tile(
                            [128, 128], F16, tag="o2", name=f"o2_{t}_{c4}_{im}"
                        )
                    o2 = o2_tiles[key]
                    if use_act:
                        nc.scalar.activation(
                            out=o2[:, r0 : r0 + 64],
                            in_=avv[:, im, 0:64],
                            func=mybir.ActivationFunctionType.Copy,
                            scale=rec[:, im : im + 1],
                        )
                    else:
                        nc.vector.tensor_scalar(
                            out=o2[:, r0 : r0 + 64],
                            in0=avv[:, im, 0:64],
                            scalar1=rec[:, im : im + 1],
                            scalar2=None,
                            op0=mybir.AluOpType.mult,
                        )

            def xbar(t, c4):
                for im in range(HPC):
                    m = 4 * c4 + im
                    nc.sync.dma_start_transpose(
                        out=oT2[:, t, 128 * m : 128 * (m + 1)],
                        in_=o2_tiles[(t, im)][:, :],
                    )

            ob_tiles = {}

            def wo_group(m, ec, pin=None):
                ps = psE.tile([128, 512], F32, tag="wo", name=f"wo_{m}_{ec}")
                for dt in range(2):
                    bi = nc.tensor.matmul(
                        ps,
                        oT2[:, dt, 128 * m : 128 * (m + 1)],
                        wo_sb[:, dt, 512 * ec : 512 * (ec + 1)],
                        start=(dt == 0),
                        stop=(dt == 1),
                    )
                    if dt == 0 and pin is not None:
                        add_dep_helper(
                            bi.ins, pin, sync=True, reason="pin W_O into exp-bound window"
                        )
                if ec == 0:
                    ob_tiles[m] = obpool.tile([128, D], F16, tag="ob", name=f"ob_{m}")
                ob = ob_tiles[m]
                nc.vector.tensor_copy(out=ob[:, 512 * ec : 512 * (ec + 1)], in_=ps)
                nc.sync.dma_start(
                    out=out[128 * m : 128 * (m + 1), 512 * ec : 512 * (ec + 1)],
                    in_=ob[:, 512 * ec : 512 * (ec + 1)],
                )

            # ---------------- prologue
            warm = recpool.tile([1, 16], F32, tag="warm", name="warm")
            nc.vector.memset(warm, 0.0)
            nc.scalar.activation(
                out=warm, in_=warm, func=mybir.ActivationFunctionType.Exp, scale=1.0
            )
            # PE p-state warm-up: the ramp-to-2.4GHz clock starts at the first
            # PE activity, so issue a few dummy matmuls on preamble-written
            # constant APs (ready at t=0, no memset dependency) while the
            # first weight/x DMAs are still in flight.  Output never read.
            pw_cap = nc.const_aps.scalar_like(1.0, qT2[:, 0, 0:1])
            for wi in range(2):
                wps = psE.tile([128, 512], F32, tag="wo", name=f"pewarm_{wi}")
                nc.tensor.matmul(
                    wps[0:1, 0:1], pw_cap, pw_cap, start=True, stop=True
                )
            nc.vector.memset(V4v[:, :, :, 64:65], 1.0)

            # prologue DMAs, gated so term A (w-hi, x-hi) can start earliest:
            # w-hi -> x-hi -> w-lo -> x-lo per stream.
            def x_tile(name):
                xt = xin.tile([128, 2, DC2, 2, 512], F8, tag="xt", name=f"xt_{name}0")
                xt_tiles[(name, 0)] = xt
                return xt

            xq0, xk0 = x_tile("q"), x_tile("k")
            nc.sync.dma_start(out=wq_sb[:, 0], in_=wqT[:, 0])
            nc.sync.dma_start(out=xq0[:, 0], in_=xqT[:, 0, 0])
            nc.sync.dma_start(out=wq_sb[:, 1], in_=wqT[:, 1])
            nc.sync.dma_start(out=xq0[:, 1], in_=xqT[:, 0, 1])
            nc.sync.dma_start(out=wk_sb[:, 0], in_=wkT[:, 0])
            nc.sync.dma_start(out=xk0[:, 0], in_=xkT[:, 0, 0])
            nc.sync.dma_start(out=wk_sb[:, 1], in_=wkT[:, 1])
            nc.sync.dma_start(out=xk0[:, 1], in_=xkT[:, 0, 1])
            nc.sync.dma_start(out=tri_sb, in_=triC[:, :, :])
            nc.sync.dma_start(out=cw_sb, in_=cwC[:, :, :])
            nc.sync.dma_start(out=wv_sb, in_=wvT[:, :, :, :, :])
            x_dma("v", 0)
            nc.sync.dma_start(out=wo_sb, in_=woT[:, :, :])

            # chunk-0 q/k: both eb-groups in the two idle psA banks with
            # term-interleaved matmuls, so the PE consumes all of x-hi
            # while x-lo is still in flight (no mid-group DMA stall)
            for name in ("q", "k"):
                xt = xt_tiles[(name, 0)]
                wsb = wq_sb if name == "q" else wk_sb
                dst = qT2 if name == "q" else kT2
                pss = [
                    psA.tile([128, 1024], F32, tag="st", name=f"pj0_{name}{g}")[
                        :, 0:512
                    ]
                    for g in range(2)
                ]
                for ti, (xh, wh) in enumerate(TERMS):
                    for dc2 in range(DC2):
                        for g in range(2):
                            nc.tensor.matmul(
                                pss[g],
                                wsb[:, wh, dc2, :, 128 * g : 128 * (g + 1)],
                                xt[:, xh, dc2, :, :],
                                start=(ti == 0 and dc2 == 0),
                                stop=(ti == 2 and dc2 == DC2 - 1),
                                perf_mode=mybir.MatmulPerfMode.DoubleRow,
                            )
                for g in range(2):
                    nc.vector.tensor_copy(out=dst[:, g, 0:512], in_=pss[g])
            for g in range(4):
                proj_group("v", 0, g)

            # ---------------- main loop
            # W_O is deferred ~2 chunks and drip-fed between attention pairs
            # (late chunks are exp-bound, so the extra PE work there is free).
            # normalize(h) runs one head late, XBAR(t) one head after that, so
            # every drain op's semaphore wait is satisfied at queue time.
            # W_O of chunk cc runs two chunks later, pinned (scheduling edge)
            # onto a head's first ST so the scheduler places it inside that
            # head's exp-bound window instead of hoisting it into early stalls.
            wo_sched = {2: [0], 3: [1, 2]}
            for c4 in range(NCH):
                proj_pending = (
                    [("q", 0), ("q", 1), ("k", 0), ("k", 1),
                     ("v", 0), ("v", 1), ("v", 2), ("v", 3)]
                    if c4 + 1 < NCH
                    else []
                )
                wo_pending = [
                    (m, ec)
                    for cc in wo_sched.get(c4, [])
                    for m in range(4 * cc, 4 * cc + 4)
                    for ec in range(2)
                ]
                npops = {8: [3, 3, 2, 0], 16: [6, 6, 4, 0]}.get(
                    len(wo_pending), [2, 2, 2, 2]
                )
                for h in range(HPC):
                    attention(h, c4)
                    for _ in range(npops[h]):
                        if wo_pending:
                            wo_group(*wo_pending.pop(0), pin=anchors[(h, c4)])
                    # stagger next chunk's x streams one per head slot so the
                    # serial DMA device never backlogs in front of an XBAR
                    if c4 + 1 < NCH and h >= 1:
                        nm_ = ("q", "k", "v")[h - 1]
                        x_dma(nm_, c4 + 1)
                        npop = 2 if h < 3 else 4
                        for _ in range(npop):
                            if proj_pending:
                                name, g = proj_pending.pop(0)
                                proj_group(name, c4 + 1, g)
                for m, ec in wo_pending:
                    wo_group(m, ec)

            # ---------------- tail: W_O of the last chunk
            for m in range(4 * (NCH - 1), 4 * NCH):
                for ec in range(2):
                    wo_group(m, ec)

    return nc


# ---------------------------------------------------------------- host side
def _split8(v):
    """fp32 array -> (hi, lo) e4m3 with v ~ hi + lo."""
    import ml_dtypes

    hi = v.astype(ml_dtypes.float8_e4m3)
    lo = (v - hi.astype(np.float32)).astype(ml_dtypes.float8_e4m3)
    return hi, lo


def _swizzle_x8(Xb):
    """[S, D] f32 -> [p, c4, hl, dc2, r, s] e4m3 (d = dc2*256 + r*128 + p)."""
    xT = np.ascontiguousarray(Xb.T)  # [D, S]
    v = xT.reshape(DC2, 2, 128, NCH, 512)  # [dc2, r, p, c4, s]
    hi, lo = _split8(v)
    hl = np.stack([hi, lo], axis=0)  # [hl, dc2, r, p, c4, s]
    return np.ascontiguousarray(hl.transpose(3, 4, 0, 1, 2, 5))


def _swizzle_w8(Wrows, scale):
    """[E, D] (rows of W) -> [p, hl, dc2, r, e] e4m3 of scale * W.T."""
    wT = np.ascontiguousarray(Wrows.T).astype(np.float32) * np.float32(scale)
    v = wT.reshape(DC2, 2, 128, E)  # [dc2, r, p, e]
    hi, lo = _split8(v)
    hl = np.stack([hi, lo], axis=0)  # [hl, dc2, r, p, e]
    return np.ascontiguousarray(hl.transpose(3, 0, 1, 2, 4))


def _swizzle_wo(Wcols):
    """[D, E] -> [p, t, e] with wo[t*128+p, e] = Wcols[e, t*128+p] / WSC_V."""
    wT = np.ascontiguousarray(Wcols.T).astype(np.float32) / np.float32(WSC_V)
    return np.ascontiguousarray(
        wT.reshape(2, 128, D).transpose(1, 0, 2)
    ).astype(np.float16)


def _build_tri():
    import ml_dtypes

    m = np.zeros((128, 2, 128), np.float32)
    r = np.arange(128)[:, None]
    c = np.arange(128)[None, :]
    m[:, 0, :] = np.where(c >= r, np.float32(NEGM), np.float32(0.0))
    m[:, 1, :] = m[:, 0, :]
    return m.astype(ml_dtypes.float8_e4m3)


def _build_cw():
    import ml_dtypes

    # paired with tri on both sub-rows: addend = 2 * (-224 * 128) = -224*256
    m = np.zeros((128, 2, 640), np.float32)
    m[0, :, 0:512] = 128.0
    for t in range(127):
        m[t + 1, :, 512 + t] = 128.0
    return m.astype(ml_dtypes.float8_e4m3)


_NC_CACHE = None


def _get_module():
    global _NC_CACHE
    if _NC_CACHE is None:
        _NC_CACHE = build_module()
    return _NC_CACHE


def _numpy_fallback(Q, K, V, W_Q, W_K, W_V, W_O, mask):
    q = (Q @ W_Q.T).reshape(B, S, H, DK).transpose(0, 2, 1, 3)
    k = (K @ W_K.T).reshape(B, S, H, DK).transpose(0, 2, 1, 3)
    v = (V @ W_V.T).reshape(B, S, H, DK).transpose(0, 2, 1, 3)
    att = np.einsum("bhqd,bhkd->bhqk", q, k)
    att = np.where(np.asarray(mask), att, np.float32(-1e15)) / np.float32(np.sqrt(DK))
    att = att - att.max(axis=3, keepdims=True)
    np.exp(att, out=att)
    att /= att.sum(axis=3, keepdims=True)
    o = np.einsum("bhqk,bhkd->bhqd", att, v)
    o = o.transpose(0, 2, 1, 3).reshape(B, S, D)
    return (o @ W_O.T).astype(np.float32)


def kernel(Q, K, V, W_Q, W_K, W_V, W_O, mask):
    from concourse.bass_utils import run_bass_kernel_spmd

    Q = np.asarray(Q, dtype=np.float32)
    K = np.asarray(K, dtype=np.float32)
    V = np.asarray(V, dtype=np.float32)
    W_Q = np.asarray(W_Q, dtype=np.float32)
    W_K = np.asarray(W_K, dtype=np.float32)
    W_V = np.asarray(W_V, dtype=np.float32)
    W_O = np.asarray(W_O, dtype=np.float32)
    mask_b = np.asarray(mask).reshape(S, S).astype(bool)
    if not np.array_equal(mask_b, np.tril(np.ones((S, S), dtype=bool))):
        return _numpy_fallback(Q, K, V, W_Q, W_K, W_V, W_O, np.asarray(mask))

    nc = _get_module()
    tri_np = _build_tri()
    cw_np = _build_cw()
    xs = {b: (_swizzle_x8(Q[b]), _swizzle_x8(K[b]), _swizzle_x8(V[b])) for b in range(B)}
    in_maps = []
    for c in range(8):
        b, g = divmod(c, 4)
        rows = slice(g * E, (g + 1) * E)
        in_maps.append(
            {
                "xqT": xs[b][0],
                "xkT": xs[b][1],
                "xvT": xs[b][2],
                "wqT": _swizzle_w8(W_Q[rows], WSC_QK),
                "wkT": _swizzle_w8(W_K[rows], WSC_QK),
                "wvT": _swizzle_w8(W_V[rows], WSC_V),
                "woT": _swizzle_wo(W_O[:, rows]),
                "triC": tri_np,
                "cwC": cw_np,
            }
        )
    res = run_bass_kernel_spmd(nc, in_maps, core_ids=list(range(8)))
    parts = [res.results[c]["out"].astype(np.float32) for c in range(8)]
    return np.stack(
        [
            parts[0] + parts[1] + parts[2] + parts[3],
            parts[4] + parts[5] + parts[6] + parts[7],
        ]
    ).astype(np.float32)

